# revision 30
# baseline (speedup 1.0000x reference)
"""Trainium2 Bass kernel for the gnn_message_passing Combiner model.

Strategy (8 NeuronCores, data-parallel over batch, sync-BN on host):
  - batch 128 split 16-per-core, processed as 8 batch-PAIRS; params replicated.
  - x host-prepped to [BL, 128p, 16ch*512c] bf16 with n = p*16 + ch so each
    per-batch DMA is one contiguous 2 MiB transfer (16 KiB per partition);
    x streams on the SP HWDGE ring, params/outputs on the Activation ring.
  - per pair (b0, b1), all-bf16 PE path, f32 PSUM, 3-stage pipeline
    [p1 | tr+conv+qk+a1-chain | bmm]:
      p1   : hsT[128(j b0|j b1), 512c] via col-tiled matmuls — b0 at array
             cols 0-63, b1 at cols 64-127, interleaved accumulation groups
             (per-partition has_written), so the two streams run concurrently
             at full 128x128 array utilization (~216 ns per chunk pair).
      tr   : 4 PE transposes [128,128] -> hs[c, (j b0|j b1)]; evac on DVE.
      conv : 4 full-M matmuls, out [128, 512] = (hs2T b0 | hs2T b1) rows.
      qk   : 4 tiny matmuls (N=2) on the hs chunks -> q1/k1 per partition.
      krep : k1 row replicated on DVE + two concurrent quadrant transposes;
             ONE pair-wide tanh on ACT; a1 = [adj + a*T | v1 | s1] on DVE.
             (tensor_tensor_reduce hangs on HW - bisected; avoided.)
      bmm  : per batch, M=64 matmul (array cols 0-63) + M=2 extra rows
             (pooled p, channel sum) at cols 64-95; the previous pair's
             sum-of-squares ones-matmul rides at cols 96-127 concurrently.
  - outputs per core: pooled pre-BN p [8, 1024], bn stats [2, 512].
  - host: combine BN stats across cores, fold BN affine into classifier,
    tiny [128,512]@[512,200] matmul in f64.
"""

import functools
import os
from contextlib import ExitStack

import numpy as np
import ml_dtypes
_BF = ml_dtypes.bfloat16

import concourse.bass as bass
from concourse import bacc
import concourse.mybir as mybir
import concourse.tile as tile
from concourse.bass_utils import run_bass_kernel_spmd

F32 = mybir.dt.float32
BF16 = mybir.dt.bfloat16

B, N, C, J, K = 128, 2048, 512, 64, 200
NCORES = 8
BL = B // NCORES          # 16 local batches
NP = BL // 2              # 8 pairs
NCH = N // 128            # 16 n-chunks
CCH = C // 128            # 4 c-chunks
BN_EPS = 1e-5
PF = 3                    # pairs of x tiles prefetched ahead of compute

LAST_RESULTS = None       # test.py reads .exec_time_ns after a traced run


def _install_ntff_hook_shim():
    """The agent image's ``antenv`` lacks ``axon_hooks``; provide it so
    run_bass_kernel_spmd(trace=True) can capture NTFF profiles via the
    libaxon_pjrt.so C ABI (same mechanism as trn_boot's installer)."""
    import contextlib
    import ctypes
    import sys
    import types

    try:
        import antenv.axon_hooks  # noqa: F401
        return
    except ImportError:
        pass

    mod = types.ModuleType("antenv.axon_hooks")
    holder = {"hook": None}
    mod.set_axon_ntff_profile_hook = lambda h: holder.__setitem__("hook", h)
    mod.get_axon_ntff_profile_hook = lambda: holder["hook"]
    sys.modules["antenv.axon_hooks"] = mod
    try:
        import antenv
        antenv.axon_hooks = mod
    except ImportError:
        pass

    so_path = "/opt/axon/libaxon_pjrt.so"
    if not os.path.exists(so_path):
        return
    try:
        lib = ctypes.CDLL(so_path)
    except OSError:
        return
    if not hasattr(lib, "axon_start_nrt_profile"):
        return
    lib.axon_start_nrt_profile.argtypes = [
        ctypes.POINTER(ctypes.c_int64), ctypes.c_size_t]
    lib.axon_start_nrt_profile.restype = ctypes.c_int64
    lib.axon_stop_nrt_profile.argtypes = [ctypes.c_char_p]
    lib.axon_stop_nrt_profile.restype = ctypes.c_int64

    @contextlib.contextmanager
    def _hook(output_dir, device_ids):
        import jax
        jax.devices()
        if device_ids:
            ids = (ctypes.c_int64 * len(device_ids))(*device_ids)
            rc = lib.axon_start_nrt_profile(ids, len(device_ids))
        else:
            rc = lib.axon_start_nrt_profile(None, 0)
        if rc != 0:
            raise RuntimeError(f"axon_start_nrt_profile rc={rc}")
        try:
            yield
        finally:
            n = lib.axon_stop_nrt_profile(str(output_dir).encode())
            if n < 0:
                raise RuntimeError(f"axon_stop_nrt_profile rc={n}")
            print(f"profile: {n} file(s) written to {output_dir}")

    mod.set_axon_ntff_profile_hook(_hook)


_install_ntff_hook_shim()

_F = lambda k: bool(int(os.environ.get(k, "0")))
SAFE_P1 = _F("SAFE_P1")        # no b0/b1 interleave in p1
SAFE_SSQ = _F("SAFE_SSQ")      # ssq at default tile position (serialized)
SAFE_BMM = _F("SAFE_BMM")      # single M=66 bmm matmul
# tensor_tensor_reduce hangs on HW (verified by bisection) — the ttr-free
# paths are permanent defaults; env can re-enable for experiments.
SAFE_CHAIN = bool(int(os.environ.get("SAFE_CHAIN", "1")))
SAFE_DMA = _F("SAFE_DMA")      # all DMAs on the SP (sync) ring
SAFE_QK = bool(int(os.environ.get("SAFE_QK", "1")))

ADD = mybir.AluOpType.add
MULT = mybir.AluOpType.mult
TANH = mybir.ActivationFunctionType.Tanh
COPY = mybir.ActivationFunctionType.Copy
SQUARE = mybir.ActivationFunctionType.Square


class _Stage:
    """Per-pair emission state shared between pipeline stages."""
    def __init__(self):
        self.hsT_bf = None
        self.hs = None
        self.hs2T = None
        self.qk = None
        self.krep2 = None
        self.tr_ps = None   # [128, 640] bf16 psum: 0:512 transposes, 512:640 krepT
        self.a1 = None
        self.sq = None


def _emit_p1(nc, pools, sb, st, xt0, xt1, first):
    """hsT pair: rows 0:64 = b0's [j, c], 64:128 = b1's.  Col-tiled: b0 on
    array cols 0-63, b1 on cols 64-127, interleaved so they run concurrently.
    For the first pair, emit b0's chunks before b1's (b1's x lands later)."""
    work, psum = pools
    ps = psum.tile([128, C], F32, tag="p1", bufs=2, name="ps_p1")
    if first or SAFE_P1:
        for ch in range(NCH):
            nc.tensor.matmul(ps[0:64, :], lhsT=sb["w0"][:, ch, :],
                             rhs=xt0[:, ch * 512:(ch + 1) * 512],
                             start=(ch == 0), stop=(ch == NCH - 1))
        for ch in range(NCH):
            nc.tensor.matmul(ps[64:128, :], lhsT=sb["w0"][:, ch, :],
                             rhs=xt1[:, ch * 512:(ch + 1) * 512],
                             start=(ch == 0), stop=(ch == NCH - 1))
    else:
        # Two interleaved accumulation groups, one per partition half of the
        # same bank: has_written clears are per written partition, so the
        # groups are independent; the sim's coarse region check is skipped.
        for ch in range(NCH):
            nc.tensor.matmul(ps[0:64, :], lhsT=sb["w0"][:, ch, :],
                             rhs=xt0[:, ch * 512:(ch + 1) * 512],
                             start=(ch == 0), stop=(ch == NCH - 1),
                             skip_group_check=True)
            nc.tensor.matmul(ps[64:128, :], lhsT=sb["w0"][:, ch, :],
                             rhs=xt1[:, ch * 512:(ch + 1) * 512],
                             start=(ch == 0), stop=(ch == NCH - 1),
                             skip_group_check=True)
    hsT_bf = work.tile([128, C], BF16, tag="hsT_bf", bufs=3, name="hsT_bf")
    nc.vector.tensor_copy(hsT_bf, ps)
    st.hsT_bf = hsT_bf
    # q1/k1 on the otherwise-idle GpSimd engine, one stage earlier than the
    # transpose path needs them -- the krepT/tanh/a1 chain then has a full
    # iteration of slack and the bmm LDWEIGHTS never stalls on it.
    qk = work.tile([128, 2], F32, tag="qk_sb", bufs=3, name="qk_sb")
    scr = work.tile([128, C], BF16, tag="scr", bufs=1, name="scr")
    scr2 = work.tile([128, C], BF16, tag="scr2", bufs=1, name="scr2")
    nc.gpsimd.tensor_tensor(scr, hsT_bf, sb["wqb"], op=MULT)
    nc.gpsimd.tensor_tensor(scr2, hsT_bf, sb["wkb"], op=MULT)
    nc.vector.tensor_reduce(qk[:, 0:1], scr, axis=mybir.AxisListType.X, op=ADD)
    nc.vector.tensor_reduce(qk[:, 1:2], scr2, axis=mybir.AxisListType.X, op=ADD)
    st.qk = qk
    krep2 = work.tile([128, J], BF16, tag="krep2", bufs=2, name="krep2")
    nc.vector.tensor_scalar(krep2, sb["ones128"][:, 0:J], qk[:, 1:2],
                            None, MULT)
    st.krep2 = krep2


def _emit_tr(nc, pools, sb, st):
    """transpose -> hs[c, (j b0 | j b1)] in 4 chunks of [128, 128] (PE)."""
    work, psum = pools
    tr_ps = psum.tile([128, 576], BF16, tag="tr", bufs=1, name="ps_tr")
    for cc in range(CCH):
        nc.tensor.transpose(tr_ps[:, cc * 128:(cc + 1) * 128],
                            in_=st.hsT_bf[:, cc * 128:(cc + 1) * 128],
                            identity=sb["ident"])
    hs = work.tile([128, C], BF16, tag="hs", bufs=3, name="hs")
    nc.vector.tensor_copy(hs, tr_ps[:, 0:512])
    st.hs = hs
    st.tr_ps = tr_ps


def _emit_convqk(nc, pools, sb, st):
    """conv1 (full-M matmuls) + qk (tiny matmuls) + tanh/a1 chain."""
    work, psum = pools
    qk, krep2 = st.qk, st.krep2
    ps = psum.tile([128, C], F32, tag="conv", bufs=1, name="ps_conv")
    for cc in range(CCH):
        nc.tensor.matmul(ps, lhsT=st.hs[:, cc * 128:(cc + 1) * 128],
                         rhs=sb["wc"][:, cc, :],
                         start=(cc == 0), stop=(cc == CCH - 1))
    hs2T = work.tile([128, C], BF16, tag="hs2T", bufs=3, name="hs2T")
    nc.vector.tensor_copy(hs2T, ps)
    st.hs2T = hs2T

    # k1 row transposes: disjoint array quadrants, run concurrently.
    nc.tensor.matmul(st.tr_ps[0:64, 512:576], lhsT=krep2[0:64, :],
                     rhs=sb["ident2"][0:64, :], is_transpose=True,
                     skip_group_check=True)
    nc.tensor.matmul(st.tr_ps[64:128, 512:576], lhsT=krep2[64:128, :],
                     rhs=sb["ident2"][64:128, :], is_transpose=True,
                     skip_group_check=True)
    t2 = work.tile([128, J], BF16, tag="t2", bufs=2, name="t2")
    nc.scalar.activation(t2, st.tr_ps[:, 512:576], TANH,
                         bias=qk[:, 0:1], scale=-1.0)
    t2a = work.tile([128, J], BF16, tag="t2a", bufs=2, name="t2a")
    nc.vector.tensor_scalar(t2a, t2, sb["alpha"], None, MULT)
    s1t = work.tile([128, 1], F32, tag="s1t", bufs=2, name="s1t")
    nc.vector.tensor_reduce(s1t, t2a, axis=mybir.AxisListType.X, op=ADD)
    a1 = work.tile([128, J + 2], BF16, tag="a1", bufs=2, name="a1")
    nc.vector.tensor_tensor(a1[:, 0:J], t2a, sb["adj"], op=ADD)
    tw = work.tile([128, J], BF16, tag="tw", bufs=1, name="tw")
    nc.vector.tensor_tensor(tw, t2a, sb["w1rep"], op=MULT)
    v1t = work.tile([128, 1], F32, tag="v1t", bufs=2, name="v1t")
    nc.vector.tensor_reduce(v1t, tw, axis=mybir.AxisListType.X, op=ADD)
    nc.vector.tensor_scalar(a1[:, J:J + 1], v1t, sb["adjv"], None, ADD)
    nc.vector.tensor_scalar(a1[:, J + 1:J + 2], s1t, sb["adjs"], None, ADD)
    st.a1 = a1


def _emit_bmm(nc, pools, sb, st, pp, st_prev, psum_ssq, p_pair):
    """Per batch of the pair: M=64 bmm (array cols 0-63) + M=2 extra rows
    (cols 64-95); the previous pair's ssq ones-matmul rides at cols 96-127.
    sq/p4/bn2 evacuations all on DVE (ACT does only tanh)."""
    work, psum = pools
    sq = work.tile([128, C], BF16, tag="sq", bufs=3, name="sq")
    for b in range(2):
        base = 64 * b
        pb = psum.tile([66, C], F32, tag="bmm", bufs=2, name="ps_bmm")
        if SAFE_BMM:
            nc.tensor.matmul(pb, lhsT=st.a1[base:base + 64, :],
                             rhs=st.hs2T[base:base + 64, :],
                             start=True, stop=True)
        else:
            nc.tensor.matmul(pb[0:64, :], lhsT=st.a1[base:base + 64, 0:64],
                             rhs=st.hs2T[base:base + 64, :],
                             start=True, stop=True)
            nc.tensor.matmul(pb[64:66, :], lhsT=st.a1[base:base + 64, 64:66],
                             rhs=st.hs2T[base:base + 64, :],
                             start=True, stop=True, skip_group_check=True)
        if b == 0 and st_prev is not None:
            _emit_ssq(nc, sb, st_prev, pp - 1, psum_ssq)
        nc.scalar.activation(sq[base:base + 64, :], pb[0:64, :], SQUARE)
        nc.scalar.activation(p_pair[:, b * C:(b + 1) * C], pb[64:65, :], COPY)
        nc.vector.tensor_tensor(sb["bn2"], sb["bn2"], pb[64:66, :], op=ADD)
    st.sq = sq


def _emit_ssq(nc, sb, st, pp, psum_ssq):
    """Accumulate sum-of-squares over (pair, j) into one PSUM row via PE,
    col-tiled at array cols 96-127 so it overlaps the adjacent bmm."""
    if SAFE_SSQ:
        nc.tensor.matmul(psum_ssq[0:1, :], lhsT=sb["onescol"], rhs=st.sq,
                         start=(pp == 0), stop=(pp == NP - 1))
    else:
        nc.tensor.matmul(psum_ssq[96:97, :], lhsT=sb["onescol"], rhs=st.sq,
                         start=(pp == 0), stop=(pp == NP - 1),
                         tile_position=(0, 96))


def _build():
    nc = bacc.Bacc("TRN2", target_bir_lowering=False)

    x = nc.dram_tensor("x", [BL, 128, NCH * 512], BF16, kind="ExternalInput")
    w0p = nc.dram_tensor("w0p", [128, NCH * J], BF16, kind="ExternalInput")
    wcp = nc.dram_tensor("wcp", [128, CCH * C], BF16, kind="ExternalInput")
    wqkb = nc.dram_tensor("wqkb", [128, 2 * C], BF16, kind="ExternalInput")
    adj2 = nc.dram_tensor("adj2", [128, J], F32, kind="ExternalInput")
    alpha2 = nc.dram_tensor("alpha2", [128, 1], F32, kind="ExternalInput")
    adjv2 = nc.dram_tensor("adjv2", [128, 1], F32, kind="ExternalInput")
    adjs2 = nc.dram_tensor("adjs2", [128, 1], F32, kind="ExternalInput")
    w1rep2 = nc.dram_tensor("w1rep2", [128, J], F32, kind="ExternalInput")

    p_out = nc.dram_tensor("p_out", [NP, 2 * C], F32, kind="ExternalOutput")
    stats_out = nc.dram_tensor("stats_out", [2, C], F32, kind="ExternalOutput")

    pdma = nc.sync if SAFE_DMA else nc.scalar

    with ExitStack() as ctx:
        tc = ctx.enter_context(tile.TileContext(nc))
        consts = ctx.enter_context(tc.tile_pool(name="consts", bufs=1))
        xpool = ctx.enter_context(tc.tile_pool(name="xpool", bufs=2 * (PF + 1)))
        work = ctx.enter_context(tc.tile_pool(name="work", bufs=2))
        psum = ctx.enter_context(tc.tile_pool(name="psum", bufs=1, space="PSUM"))

        # ---- constants on the Activation HWDGE ring (parallel to x) ----
        w0_sb = consts.tile([128, NCH, J], BF16, name="w0_sb")
        pdma.dma_start(out=w0_sb, in_=w0p.rearrange("p (t j) -> p t j", j=J))
        ident_dram = nc.inline_tensor(
            np.eye(128, dtype=np.float32).astype(_BF), name="ident128")
        ident = consts.tile([128, 128], BF16, name="ident")
        pdma.dma_start(out=ident, in_=ident_dram[:, :])
        wqkb_sb = consts.tile([128, 2, C], BF16, name="wqkb_sb")
        pdma.dma_start(out=wqkb_sb,
                       in_=wqkb.rearrange("p (s c) -> p s c", c=C))
        ident2_dram = nc.inline_tensor(
            np.tile(np.eye(J, dtype=np.float32), (2, 1)).astype(_BF),
            name="identj2")
        ident2 = consts.tile([128, J], BF16, name="ident2")
        pdma.dma_start(out=ident2, in_=ident2_dram[:, :])
        adj_sb = consts.tile([128, J], F32, name="adj_sb")
        pdma.dma_start(out=adj_sb, in_=adj2[:, :])
        alpha_sb = consts.tile([128, 1], F32, name="alpha_sb")
        pdma.dma_start(out=alpha_sb, in_=alpha2[:, :])
        adjv_sb = consts.tile([128, 1], F32, name="adjv_sb")
        pdma.dma_start(out=adjv_sb, in_=adjv2[:, :])
        adjs_sb = consts.tile([128, 1], F32, name="adjs_sb")
        pdma.dma_start(out=adjs_sb, in_=adjs2[:, :])
        w1rep_sb = consts.tile([128, J], F32, name="w1rep_sb")
        pdma.dma_start(out=w1rep_sb, in_=w1rep2[:, :])
        wc_sb = consts.tile([128, CCH, C], BF16, name="wc_sb")
        pdma.dma_start(out=wc_sb, in_=wcp.rearrange("p (q o) -> p q o", o=C))

        ones128 = consts.tile([128, 128], BF16, name="ones128")
        nc.vector.memset(ones128, 1.0)
        onescol = consts.tile([128, 1], BF16, name="onescol")
        nc.vector.memset(onescol, 1.0)
        bn2 = consts.tile([2, C], F32, name="bn2")
        nc.vector.memset(bn2, 0.0)

        # ---- x prefetch ring on the SP ring (one 2 MiB DMA per batch;
        # the first two batches split finer to shorten the ramp) ----
        xts = {}

        def load_x(b):
            if b < BL:
                xt = xpool.tile([128, NCH * 512], BF16, tag="xt", name="xt")
                nsplit = 8 if b == 0 else (4 if b == 1 else 1)
                q = NCH * 512 // nsplit
                for i in range(nsplit):
                    nc.sync.dma_start(out=xt[:, i * q:(i + 1) * q],
                                      in_=x[b, :, i * q:(i + 1) * q])
                xts[b] = xt

        for b in range(2 * PF):
            load_x(b)

        sb = dict(w0=w0_sb, wc=wc_sb, wqb=wqkb_sb[:, 0, :],
                  wkb=wqkb_sb[:, 1, :], ident2=ident2,
                  adj=adj_sb, alpha=alpha_sb, adjv=adjv_sb, adjs=adjs_sb,
                  w1rep=w1rep_sb, ident=ident, ones128=ones128,
                  onescol=onescol, bn2=bn2)
        pools = (work, psum)

        psum_ssq = psum.tile([128, C], F32, tag="ssq", bufs=1, name="ps_ssq")
        stages = [_Stage() for _ in range(NP)]
        p_pairs = [None] * NP

        def emit_bmm_stage(pp):
            p_pairs[pp] = pp_t = work.tile([1, 2 * C], F32, tag="p_pair",
                                           bufs=2, name="p_pair")
            _emit_bmm(nc, pools, sb, stages[pp], pp,
                      stages[pp - 1] if pp >= 1 else None, psum_ssq, pp_t)
            pdma.dma_start(out=p_out[pp:pp + 1, :], in_=pp_t)

        for i in range(NP):
            load_x(2 * (i + PF))
            load_x(2 * (i + PF) + 1)
            _emit_p1(nc, pools, sb, stages[i], xts.pop(2 * i),
                     xts.pop(2 * i + 1), first=(i == 0))
            if i >= 2:
                emit_bmm_stage(i - 2)
            if i >= 1:
                _emit_tr(nc, pools, sb, stages[i - 1])
                _emit_convqk(nc, pools, sb, stages[i - 1])

        # drain
        _emit_tr(nc, pools, sb, stages[NP - 1])
        _emit_convqk(nc, pools, sb, stages[NP - 1])
        emit_bmm_stage(NP - 2)
        emit_bmm_stage(NP - 1)
        _emit_ssq(nc, sb, stages[NP - 1], NP - 1, psum_ssq)

        ssq_row = 0 if SAFE_SSQ else 96
        ssq_sb = consts.tile([97, C], F32, name="ssq_sb")
        nc.vector.tensor_copy(ssq_sb[ssq_row:ssq_row + 1, :],
                              psum_ssq[ssq_row:ssq_row + 1, :])
        pdma.dma_start(out=stats_out[0:1, :], in_=bn2[1:2, :])
        pdma.dma_start(out=stats_out[1:2, :],
                            in_=ssq_sb[ssq_row:ssq_row + 1, :])

    nc.compile()
    return nc


@functools.lru_cache(maxsize=1)
def _built():
    return _build()


def _prep_params(inputs):
    f = lambda a: np.ascontiguousarray(np.asarray(a, dtype=np.float32))
    w_pool0 = f(inputs["w_pool0"])                       # [J, N]
    w0p = np.ascontiguousarray(
        w_pool0.reshape(J, 128, NCH).transpose(1, 2, 0)  # [p, ch, j]
    ).reshape(128, NCH * J).astype(_BF)
    w_conv1 = f(inputs["w_conv1"])                       # [O, C]
    wcp = np.ascontiguousarray(
        w_conv1.T.reshape(CCH, 128, C).transpose(1, 0, 2)  # [p, cc, o]
    ).reshape(128, CCH * C).astype(_BF)
    w_q, w_k = f(inputs["w_q"]), f(inputs["w_k"])
    wqb = np.tile(w_q.mean(axis=0)[None, :], (128, 1))
    wkb = np.tile(w_k.mean(axis=0)[None, :], (128, 1))
    wqkb = np.concatenate([wqb, wkb], axis=1).astype(_BF)  # [128, 2C]
    adj1 = np.asarray(inputs["adj1"], np.float64)
    w1 = np.asarray(inputs["w_pool1"], np.float64).reshape(J)
    t2 = lambda a: np.ascontiguousarray(np.tile(a, (2, 1))).astype(np.float32)
    params = {
        "w0p": w0p, "wcp": wcp, "wqkb": wqkb,
        "adj2": t2(np.asarray(inputs["adj1"], np.float32)),
        "alpha2": np.full((128, 1),
                          np.asarray(inputs["alpha1"]).reshape(-1)[0],
                          np.float32),
        "adjv2": t2((adj1 @ w1).astype(np.float32).reshape(J, 1)),
        "adjs2": t2(adj1.sum(axis=1).astype(np.float32).reshape(J, 1)),
        "w1rep2": np.tile(w1.astype(np.float32)[None, :], (128, 1)),
    }
    return params


def _biases_zero(inputs):
    return all(np.abs(np.asarray(inputs[k])).max() < 1e-30
               for k in ("b_pool0", "b_conv1", "b_q", "b_k"))


def _numpy_reference(inputs):
    """Exact fallback (host) for the general nonzero-bias case."""
    g = lambda a: np.asarray(a, np.float64)
    x = g(inputs["x"]); w_pool0 = g(inputs["w_pool0"]); b_pool0 = g(inputs["b_pool0"])
    adj1 = g(inputs["adj1"]); w_conv1 = g(inputs["w_conv1"]); b_conv1 = g(inputs["b_conv1"])
    w_q = g(inputs["w_q"]); b_q = g(inputs["b_q"])
    w_k = g(inputs["w_k"]); b_k = g(inputs["b_k"])
    alpha1 = float(g(inputs["alpha1"]).reshape(-1)[0])
    gamma = g(inputs["gamma"]); beta = g(inputs["beta"])
    w_pool1 = g(inputs["w_pool1"]); b_pool1 = float(g(inputs["b_pool1"]).reshape(-1)[0])
    w_cls = g(inputs["w_cls"]); b_cls = g(inputs["b_cls"])
    hs = np.einsum("bnc,jn->bcj", x, w_pool0) + b_pool0
    q1 = (np.einsum("bcj,qc->bqj", hs, w_q) + b_q[None, :, None]).mean(axis=1)
    k1 = (np.einsum("bcj,qc->bqj", hs, w_k) + b_k[None, :, None]).mean(axis=1)
    A1 = adj1 + np.tanh(q1[:, :, None] - k1[:, None, :]) * alpha1
    hs = np.einsum("bcj,oc->boj", hs, w_conv1) + b_conv1[None, :, None]
    hs = np.einsum("bcj,bjk->bck", hs, A1)
    mean = hs.mean(axis=(0, 2), keepdims=True)
    var = hs.var(axis=(0, 2), keepdims=True)
    hs = (hs - mean) / np.sqrt(var + BN_EPS)
    hs = hs * gamma[None, :, None] + beta[None, :, None]
    hs = (np.einsum("bcj,oj->bco", hs, w_pool1) + b_pool1).reshape(hs.shape[0], -1)
    return (hs @ w_cls.T + b_cls).astype(np.float32)


def kernel(**inputs) -> np.ndarray:
    global LAST_RESULTS
    x = np.ascontiguousarray(np.asarray(inputs["x"], dtype=np.float32))
    assert x.shape == (B, N, C), x.shape
    if not _biases_zero(inputs):
        return _numpy_reference(inputs)
    # n = p*16 + ch layout: x[b].reshape(128, 16, 512) is already [p, ch, c]
    x_bf = x.astype(_BF).reshape(B, 128, NCH * 512)
    params = _prep_params(inputs)

    nc = _built()
    in_maps = []
    for core in range(NCORES):
        m = {"x": x_bf[core * BL:(core + 1) * BL]}
        m.update(params)
        in_maps.append(m)

    trace = bool(int(os.environ.get("KERNEL_TRACE", "0")))
    res = run_bass_kernel_spmd(nc, in_maps, core_ids=list(range(NCORES)),
                               trace=trace)
    LAST_RESULTS = res

    p = np.zeros((B, C), np.float64)
    bn_sum = np.zeros(C, np.float64)
    bn_ssq = np.zeros(C, np.float64)
    for core in range(NCORES):
        out = res.results[core]
        p[core * BL:(core + 1) * BL] = np.asarray(
            out["p_out"], np.float64).reshape(BL, C)
        stats = np.asarray(out["stats_out"], np.float64)   # [2, C]
        bn_sum += stats[0]
        bn_ssq += stats[1]

    gamma = np.asarray(inputs["gamma"], np.float64)
    beta = np.asarray(inputs["beta"], np.float64)
    w1 = np.asarray(inputs["w_pool1"], np.float64)[0]
    b_pool1 = float(np.asarray(inputs["b_pool1"]).reshape(-1)[0])
    w_cls = np.asarray(inputs["w_cls"], np.float64)
    b_cls = np.asarray(inputs["b_cls"], np.float64)

    cnt = B * J
    mu = bn_sum / cnt
    var = bn_ssq / cnt - mu ** 2
    r = 1.0 / np.sqrt(var + BN_EPS)
    a = gamma * r
    S = w1.sum()
    d = beta * S + b_pool1 - a * mu * S
    out = (p * a[None, :]) @ w_cls.T + (w_cls @ d + b_cls)[None, :]
    return out.astype(np.float32)


# revision 31
# speedup vs baseline: 1.0372x; 1.0372x over previous
"""Trainium2 Bass kernel for the gnn_message_passing Combiner model.

Strategy (8 NeuronCores, data-parallel over batch, sync-BN on host):
  - batch 128 split 16-per-core, processed as 8 batch-PAIRS; params replicated.
  - x host-prepped to [BL, 128p, 16ch*512c] bf16 with n = p*16 + ch so each
    per-batch DMA is one contiguous 2 MiB transfer (16 KiB per partition);
    x streams on the SP HWDGE ring, params/outputs on the Activation ring.
  - per pair (b0, b1), all-bf16 PE path, f32 PSUM, 3-stage pipeline
    [p1 | tr+conv+qk+a1-chain | bmm]:
      p1   : hsT[128(j b0|j b1), 512c] via col-tiled matmuls — b0 at array
             cols 0-63, b1 at cols 64-127, interleaved accumulation groups
             (per-partition has_written), so the two streams run concurrently
             at full 128x128 array utilization (~216 ns per chunk pair).
      tr   : 4 PE transposes [128,128] -> hs[c, (j b0|j b1)]; evac on DVE.
      conv : 4 full-M matmuls, out [128, 512] = (hs2T b0 | hs2T b1) rows.
      qk   : 4 tiny matmuls (N=2) on the hs chunks -> q1/k1 per partition.
      krep : k1 row replicated on DVE + two concurrent quadrant transposes;
             ONE pair-wide tanh on ACT; a1 = [adj + a*T | v1 | s1] on DVE.
             (tensor_tensor_reduce hangs on HW - bisected; avoided.)
      bmm  : per batch, M=64 matmul (array cols 0-63) + M=2 extra rows
             (pooled p, channel sum) at cols 64-95; the previous pair's
             sum-of-squares ones-matmul rides at cols 96-127 concurrently.
  - outputs per core: pooled pre-BN p [8, 1024], bn stats [2, 512].
  - host: combine BN stats across cores, fold BN affine into classifier,
    tiny [128,512]@[512,200] matmul in f64.
"""

import functools
import os
from contextlib import ExitStack

import numpy as np
import ml_dtypes
_BF = ml_dtypes.bfloat16

import concourse.bass as bass
from concourse import bacc
import concourse.mybir as mybir
import concourse.tile as tile
from concourse.bass_utils import run_bass_kernel_spmd

F32 = mybir.dt.float32
BF16 = mybir.dt.bfloat16

B, N, C, J, K = 128, 2048, 512, 64, 200
NCORES = 8
BL = B // NCORES          # 16 local batches
NP = BL // 2              # 8 pairs
NCH = N // 128            # 16 n-chunks
CCH = C // 128            # 4 c-chunks
BN_EPS = 1e-5
PF = 3                    # pairs of x tiles prefetched ahead of compute

LAST_RESULTS = None       # test.py reads .exec_time_ns after a traced run


def _install_ntff_hook_shim():
    """The agent image's ``antenv`` lacks ``axon_hooks``; provide it so
    run_bass_kernel_spmd(trace=True) can capture NTFF profiles via the
    libaxon_pjrt.so C ABI (same mechanism as trn_boot's installer)."""
    import contextlib
    import ctypes
    import sys
    import types

    try:
        import antenv.axon_hooks  # noqa: F401
        return
    except ImportError:
        pass

    mod = types.ModuleType("antenv.axon_hooks")
    holder = {"hook": None}
    mod.set_axon_ntff_profile_hook = lambda h: holder.__setitem__("hook", h)
    mod.get_axon_ntff_profile_hook = lambda: holder["hook"]
    sys.modules["antenv.axon_hooks"] = mod
    try:
        import antenv
        antenv.axon_hooks = mod
    except ImportError:
        pass

    so_path = "/opt/axon/libaxon_pjrt.so"
    if not os.path.exists(so_path):
        return
    try:
        lib = ctypes.CDLL(so_path)
    except OSError:
        return
    if not hasattr(lib, "axon_start_nrt_profile"):
        return
    lib.axon_start_nrt_profile.argtypes = [
        ctypes.POINTER(ctypes.c_int64), ctypes.c_size_t]
    lib.axon_start_nrt_profile.restype = ctypes.c_int64
    lib.axon_stop_nrt_profile.argtypes = [ctypes.c_char_p]
    lib.axon_stop_nrt_profile.restype = ctypes.c_int64

    @contextlib.contextmanager
    def _hook(output_dir, device_ids):
        import jax
        jax.devices()
        if device_ids:
            ids = (ctypes.c_int64 * len(device_ids))(*device_ids)
            rc = lib.axon_start_nrt_profile(ids, len(device_ids))
        else:
            rc = lib.axon_start_nrt_profile(None, 0)
        if rc != 0:
            raise RuntimeError(f"axon_start_nrt_profile rc={rc}")
        try:
            yield
        finally:
            n = lib.axon_stop_nrt_profile(str(output_dir).encode())
            if n < 0:
                raise RuntimeError(f"axon_stop_nrt_profile rc={n}")
            print(f"profile: {n} file(s) written to {output_dir}")

    mod.set_axon_ntff_profile_hook(_hook)


_install_ntff_hook_shim()

_F = lambda k: bool(int(os.environ.get(k, "0")))
SAFE_P1 = _F("SAFE_P1")        # no b0/b1 interleave in p1
SAFE_SSQ = _F("SAFE_SSQ")      # ssq at default tile position (serialized)
SAFE_BMM = _F("SAFE_BMM")      # single M=66 bmm matmul
# tensor_tensor_reduce hangs on HW (verified by bisection) — the ttr-free
# paths are permanent defaults; env can re-enable for experiments.
SAFE_CHAIN = bool(int(os.environ.get("SAFE_CHAIN", "1")))
SAFE_DMA = _F("SAFE_DMA")      # all DMAs on the SP (sync) ring
SAFE_QK = bool(int(os.environ.get("SAFE_QK", "1")))

ADD = mybir.AluOpType.add
MULT = mybir.AluOpType.mult
TANH = mybir.ActivationFunctionType.Tanh
COPY = mybir.ActivationFunctionType.Copy
SQUARE = mybir.ActivationFunctionType.Square


class _Stage:
    """Per-pair emission state shared between pipeline stages."""
    def __init__(self):
        self.hsT_bf = None
        self.hs = None
        self.hs2T = None
        self.qk = None
        self.tr_ps = None   # [128, 640] bf16 psum: 0:512 transposes, 512:640 krepT
        self.a1 = None
        self.sq = None


def _emit_p1(nc, pools, sb, st, xt0, xt1, first):
    """hsT pair: rows 0:64 = b0's [j, c], 64:128 = b1's.  Col-tiled: b0 on
    array cols 0-63, b1 on cols 64-127, interleaved so they run concurrently.
    For the first pair, emit b0's chunks before b1's (b1's x lands later)."""
    work, psum = pools
    ps = psum.tile([128, C], F32, tag="p1", bufs=2, name="ps_p1")
    if first or SAFE_P1:
        for ch in range(NCH):
            nc.tensor.matmul(ps[0:64, :], lhsT=sb["w0"][:, ch, :],
                             rhs=xt0[:, ch * 512:(ch + 1) * 512],
                             start=(ch == 0), stop=(ch == NCH - 1))
        for ch in range(NCH):
            nc.tensor.matmul(ps[64:128, :], lhsT=sb["w0"][:, ch, :],
                             rhs=xt1[:, ch * 512:(ch + 1) * 512],
                             start=(ch == 0), stop=(ch == NCH - 1))
    else:
        # Two interleaved accumulation groups, one per partition half of the
        # same bank: has_written clears are per written partition, so the
        # groups are independent; the sim's coarse region check is skipped.
        for ch in range(NCH):
            nc.tensor.matmul(ps[0:64, :], lhsT=sb["w0"][:, ch, :],
                             rhs=xt0[:, ch * 512:(ch + 1) * 512],
                             start=(ch == 0), stop=(ch == NCH - 1),
                             skip_group_check=True)
            nc.tensor.matmul(ps[64:128, :], lhsT=sb["w0"][:, ch, :],
                             rhs=xt1[:, ch * 512:(ch + 1) * 512],
                             start=(ch == 0), stop=(ch == NCH - 1),
                             skip_group_check=True)
    hsT_bf = work.tile([128, C], BF16, tag="hsT_bf", bufs=3, name="hsT_bf")
    nc.vector.tensor_copy(hsT_bf, ps)
    st.hsT_bf = hsT_bf


def _emit_tr(nc, pools, sb, st):
    """transpose -> hs[c, (j b0 | j b1)] in 4 chunks of [128, 128] (PE)."""
    work, psum = pools
    tr_ps = psum.tile([128, 576], BF16, tag="tr", bufs=1, name="ps_tr")
    for cc in range(CCH):
        nc.tensor.transpose(tr_ps[:, cc * 128:(cc + 1) * 128],
                            in_=st.hsT_bf[:, cc * 128:(cc + 1) * 128],
                            identity=sb["ident"])
    hs = work.tile([128, C], BF16, tag="hs", bufs=3, name="hs")
    nc.vector.tensor_copy(hs, tr_ps[:, 0:512])
    st.hs = hs
    st.tr_ps = tr_ps


def _emit_convqk(nc, pools, sb, st):
    """conv1 (full-M matmuls) + qk (tiny matmuls) + tanh/a1 chain."""
    work, psum = pools
    # qk first: its DVE evac + krep2 build run while the PE does conv,
    # so the krepT LDWEIGHTS (stationary = krep2) doesn't stall the PE.
    ps_qk = psum.tile([128, 2], F32, tag="qk", bufs=1, name="ps_qk")
    for cc in range(CCH):
        nc.tensor.matmul(ps_qk, lhsT=st.hs[:, cc * 128:(cc + 1) * 128],
                         rhs=sb["wqk"][:, cc, :],
                         start=(cc == 0), stop=(cc == CCH - 1))
    qk = work.tile([128, 2], F32, tag="qk_sb", bufs=2, name="qk_sb")
    nc.vector.tensor_copy(qk, ps_qk)
    st.qk = qk
    krep2 = work.tile([128, J], BF16, tag="krep2", bufs=2, name="krep2")
    nc.vector.tensor_scalar(krep2, sb["ones128"][:, 0:J], qk[:, 1:2],
                            None, MULT)

    ps = psum.tile([128, C], F32, tag="conv", bufs=1, name="ps_conv")
    for cc in range(CCH):
        nc.tensor.matmul(ps, lhsT=st.hs[:, cc * 128:(cc + 1) * 128],
                         rhs=sb["wc"][:, cc, :],
                         start=(cc == 0), stop=(cc == CCH - 1))
    hs2T = work.tile([128, C], BF16, tag="hs2T", bufs=3, name="hs2T")
    nc.vector.tensor_copy(hs2T, ps)
    st.hs2T = hs2T

    # k1 row transposes: disjoint array quadrants, run concurrently.
    nc.tensor.matmul(st.tr_ps[0:64, 512:576], lhsT=krep2[0:64, :],
                     rhs=sb["ident2"][0:64, :], is_transpose=True,
                     skip_group_check=True)
    nc.tensor.matmul(st.tr_ps[64:128, 512:576], lhsT=krep2[64:128, :],
                     rhs=sb["ident2"][64:128, :], is_transpose=True,
                     skip_group_check=True)
    t2 = work.tile([128, J], BF16, tag="t2", bufs=2, name="t2")
    nc.scalar.activation(t2, st.tr_ps[:, 512:576], TANH,
                         bias=qk[:, 0:1], scale=-1.0)
    t2a = work.tile([128, J], BF16, tag="t2a", bufs=2, name="t2a")
    nc.vector.tensor_scalar(t2a, t2, sb["alpha"], None, MULT)
    s1t = work.tile([128, 1], F32, tag="s1t", bufs=2, name="s1t")
    nc.vector.tensor_reduce(s1t, t2a, axis=mybir.AxisListType.X, op=ADD)
    a1 = work.tile([128, J + 2], BF16, tag="a1", bufs=2, name="a1")
    nc.vector.tensor_tensor(a1[:, 0:J], t2a, sb["adj"], op=ADD)
    tw = work.tile([128, J], BF16, tag="tw", bufs=1, name="tw")
    nc.vector.tensor_tensor(tw, t2a, sb["w1rep"], op=MULT)
    v1t = work.tile([128, 1], F32, tag="v1t", bufs=2, name="v1t")
    nc.vector.tensor_reduce(v1t, tw, axis=mybir.AxisListType.X, op=ADD)
    nc.vector.tensor_scalar(a1[:, J:J + 1], v1t, sb["adjv"], None, ADD)
    nc.vector.tensor_scalar(a1[:, J + 1:J + 2], s1t, sb["adjs"], None, ADD)
    st.a1 = a1


def _emit_bmm(nc, pools, sb, st, pp, st_prev, psum_ssq, p_pair):
    """Per batch of the pair: M=64 bmm (array cols 0-63) + M=2 extra rows
    (cols 64-95); the previous pair's ssq ones-matmul rides at cols 96-127.
    sq/p4/bn2 evacuations all on DVE (ACT does only tanh)."""
    work, psum = pools
    sq = work.tile([128, C], BF16, tag="sq", bufs=3, name="sq")
    for b in range(2):
        base = 64 * b
        pb = psum.tile([66, C], F32, tag="bmm", bufs=2, name="ps_bmm")
        if SAFE_BMM:
            nc.tensor.matmul(pb, lhsT=st.a1[base:base + 64, :],
                             rhs=st.hs2T[base:base + 64, :],
                             start=True, stop=True)
        else:
            nc.tensor.matmul(pb[0:64, :], lhsT=st.a1[base:base + 64, 0:64],
                             rhs=st.hs2T[base:base + 64, :],
                             start=True, stop=True)
            nc.tensor.matmul(pb[64:66, :], lhsT=st.a1[base:base + 64, 64:66],
                             rhs=st.hs2T[base:base + 64, :],
                             start=True, stop=True, skip_group_check=True)
        if b == 0 and st_prev is not None:
            _emit_ssq(nc, sb, st_prev, pp - 1, psum_ssq)
        nc.scalar.activation(sq[base:base + 64, :], pb[0:64, :], SQUARE)
        nc.scalar.activation(p_pair[:, b * C:(b + 1) * C], pb[64:65, :], COPY)
        nc.vector.tensor_tensor(sb["bn2"], sb["bn2"], pb[64:66, :], op=ADD)
    st.sq = sq


def _emit_ssq(nc, sb, st, pp, psum_ssq):
    """Accumulate sum-of-squares over (pair, j) into one PSUM row via PE,
    col-tiled at array cols 96-127 so it overlaps the adjacent bmm."""
    if SAFE_SSQ:
        nc.tensor.matmul(psum_ssq[0:1, :], lhsT=sb["onescol"], rhs=st.sq,
                         start=(pp == 0), stop=(pp == NP - 1))
    else:
        nc.tensor.matmul(psum_ssq[96:97, :], lhsT=sb["onescol"], rhs=st.sq,
                         start=(pp == 0), stop=(pp == NP - 1),
                         tile_position=(0, 96))


def _build():
    nc = bacc.Bacc("TRN2", target_bir_lowering=False)

    x = nc.dram_tensor("x", [BL, 128, NCH * 512], BF16, kind="ExternalInput")
    w0p = nc.dram_tensor("w0p", [128, NCH * J], BF16, kind="ExternalInput")
    wcp = nc.dram_tensor("wcp", [128, CCH * C], BF16, kind="ExternalInput")
    wqkp = nc.dram_tensor("wqkp", [128, CCH * 2], BF16, kind="ExternalInput")
    adj2 = nc.dram_tensor("adj2", [128, J], F32, kind="ExternalInput")
    alpha2 = nc.dram_tensor("alpha2", [128, 1], F32, kind="ExternalInput")
    adjv2 = nc.dram_tensor("adjv2", [128, 1], F32, kind="ExternalInput")
    adjs2 = nc.dram_tensor("adjs2", [128, 1], F32, kind="ExternalInput")
    w1rep2 = nc.dram_tensor("w1rep2", [128, J], F32, kind="ExternalInput")

    p_out = nc.dram_tensor("p_out", [NP, 2 * C], F32, kind="ExternalOutput")
    stats_out = nc.dram_tensor("stats_out", [2, C], F32, kind="ExternalOutput")

    pdma = nc.sync if SAFE_DMA else nc.scalar

    with ExitStack() as ctx:
        tc = ctx.enter_context(tile.TileContext(nc))
        consts = ctx.enter_context(tc.tile_pool(name="consts", bufs=1))
        xpool = ctx.enter_context(tc.tile_pool(name="xpool", bufs=2 * (PF + 1)))
        work = ctx.enter_context(tc.tile_pool(name="work", bufs=2))
        psum = ctx.enter_context(tc.tile_pool(name="psum", bufs=1, space="PSUM"))

        # ---- constants on the Activation HWDGE ring (parallel to x) ----
        w0_sb = consts.tile([128, NCH, J], BF16, name="w0_sb")
        pdma.dma_start(out=w0_sb, in_=w0p.rearrange("p (t j) -> p t j", j=J))
        ident_dram = nc.inline_tensor(
            np.eye(128, dtype=np.float32).astype(_BF), name="ident128")
        ident = consts.tile([128, 128], BF16, name="ident")
        pdma.dma_start(out=ident, in_=ident_dram[:, :])
        wqk_sb = consts.tile([128, CCH, 2], BF16, name="wqk_sb")
        pdma.dma_start(out=wqk_sb, in_=wqkp.rearrange("p (q s) -> p q s", s=2))
        ident2_dram = nc.inline_tensor(
            np.tile(np.eye(J, dtype=np.float32), (2, 1)).astype(_BF),
            name="identj2")
        ident2 = consts.tile([128, J], BF16, name="ident2")
        pdma.dma_start(out=ident2, in_=ident2_dram[:, :])
        adj_sb = consts.tile([128, J], F32, name="adj_sb")
        pdma.dma_start(out=adj_sb, in_=adj2[:, :])
        alpha_sb = consts.tile([128, 1], F32, name="alpha_sb")
        pdma.dma_start(out=alpha_sb, in_=alpha2[:, :])
        adjv_sb = consts.tile([128, 1], F32, name="adjv_sb")
        pdma.dma_start(out=adjv_sb, in_=adjv2[:, :])
        adjs_sb = consts.tile([128, 1], F32, name="adjs_sb")
        pdma.dma_start(out=adjs_sb, in_=adjs2[:, :])
        w1rep_sb = consts.tile([128, J], F32, name="w1rep_sb")
        pdma.dma_start(out=w1rep_sb, in_=w1rep2[:, :])
        wc_sb = consts.tile([128, CCH, C], BF16, name="wc_sb")
        pdma.dma_start(out=wc_sb, in_=wcp.rearrange("p (q o) -> p q o", o=C))

        ones128 = consts.tile([128, 128], BF16, name="ones128")
        nc.vector.memset(ones128, 1.0)
        onescol = consts.tile([128, 1], BF16, name="onescol")
        nc.vector.memset(onescol, 1.0)
        bn2 = consts.tile([2, C], F32, name="bn2")
        nc.vector.memset(bn2, 0.0)

        # ---- x prefetch ring on the SP ring (one 2 MiB DMA per batch;
        # the first two batches split finer to shorten the ramp) ----
        xts = {}

        def load_x(b):
            if b < BL:
                xt = xpool.tile([128, NCH * 512], BF16, tag="xt", name="xt")
                nsplit = 8 if b == 0 else (4 if b == 1 else 1)
                q = NCH * 512 // nsplit
                for i in range(nsplit):
                    nc.sync.dma_start(out=xt[:, i * q:(i + 1) * q],
                                      in_=x[b, :, i * q:(i + 1) * q])
                xts[b] = xt

        for b in range(2 * PF):
            load_x(b)

        sb = dict(w0=w0_sb, wc=wc_sb, wqk=wqk_sb, ident2=ident2,
                  adj=adj_sb, alpha=alpha_sb, adjv=adjv_sb, adjs=adjs_sb,
                  w1rep=w1rep_sb, ident=ident, ones128=ones128,
                  onescol=onescol, bn2=bn2)
        pools = (work, psum)

        psum_ssq = psum.tile([128, C], F32, tag="ssq", bufs=1, name="ps_ssq")
        stages = [_Stage() for _ in range(NP)]
        p_pairs = [None] * NP

        def emit_bmm_stage(pp):
            p_pairs[pp] = pp_t = work.tile([1, 2 * C], F32, tag="p_pair",
                                           bufs=2, name="p_pair")
            _emit_bmm(nc, pools, sb, stages[pp], pp,
                      stages[pp - 1] if pp >= 1 else None, psum_ssq, pp_t)
            pdma.dma_start(out=p_out[pp:pp + 1, :], in_=pp_t)

        for i in range(NP):
            load_x(2 * (i + PF))
            load_x(2 * (i + PF) + 1)
            _emit_p1(nc, pools, sb, stages[i], xts.pop(2 * i),
                     xts.pop(2 * i + 1), first=(i == 0))
            if i >= 2:
                emit_bmm_stage(i - 2)
            if i >= 1:
                _emit_tr(nc, pools, sb, stages[i - 1])
                _emit_convqk(nc, pools, sb, stages[i - 1])

        # drain
        _emit_tr(nc, pools, sb, stages[NP - 1])
        _emit_convqk(nc, pools, sb, stages[NP - 1])
        emit_bmm_stage(NP - 2)
        emit_bmm_stage(NP - 1)
        _emit_ssq(nc, sb, stages[NP - 1], NP - 1, psum_ssq)

        ssq_row = 0 if SAFE_SSQ else 96
        ssq_sb = consts.tile([97, C], F32, name="ssq_sb")
        nc.vector.tensor_copy(ssq_sb[ssq_row:ssq_row + 1, :],
                              psum_ssq[ssq_row:ssq_row + 1, :])
        pdma.dma_start(out=stats_out[0:1, :], in_=bn2[1:2, :])
        pdma.dma_start(out=stats_out[1:2, :],
                            in_=ssq_sb[ssq_row:ssq_row + 1, :])

    nc.compile()
    return nc


@functools.lru_cache(maxsize=1)
def _built():
    return _build()


def _prep_params(inputs):
    f = lambda a: np.ascontiguousarray(np.asarray(a, dtype=np.float32))
    w_pool0 = f(inputs["w_pool0"])                       # [J, N]
    w0p = np.ascontiguousarray(
        w_pool0.reshape(J, 128, NCH).transpose(1, 2, 0)  # [p, ch, j]
    ).reshape(128, NCH * J).astype(_BF)
    w_conv1 = f(inputs["w_conv1"])                       # [O, C]
    wcp = np.ascontiguousarray(
        w_conv1.T.reshape(CCH, 128, C).transpose(1, 0, 2)  # [p, cc, o]
    ).reshape(128, CCH * C).astype(_BF)
    w_q, w_k = f(inputs["w_q"]), f(inputs["w_k"])
    wqk = np.stack([w_q.mean(axis=0), w_k.mean(axis=0)], axis=1)  # [C, 2]
    wqkp = np.ascontiguousarray(
        wqk.reshape(CCH, 128, 2).transpose(1, 0, 2)
    ).reshape(128, CCH * 2).astype(_BF)
    adj1 = np.asarray(inputs["adj1"], np.float64)
    w1 = np.asarray(inputs["w_pool1"], np.float64).reshape(J)
    t2 = lambda a: np.ascontiguousarray(np.tile(a, (2, 1))).astype(np.float32)
    params = {
        "w0p": w0p, "wcp": wcp, "wqkp": wqkp,
        "adj2": t2(np.asarray(inputs["adj1"], np.float32)),
        "alpha2": np.full((128, 1),
                          np.asarray(inputs["alpha1"]).reshape(-1)[0],
                          np.float32),
        "adjv2": t2((adj1 @ w1).astype(np.float32).reshape(J, 1)),
        "adjs2": t2(adj1.sum(axis=1).astype(np.float32).reshape(J, 1)),
        "w1rep2": np.tile(w1.astype(np.float32)[None, :], (128, 1)),
    }
    return params


def _biases_zero(inputs):
    return all(np.abs(np.asarray(inputs[k])).max() < 1e-30
               for k in ("b_pool0", "b_conv1", "b_q", "b_k"))


def _numpy_reference(inputs):
    """Exact fallback (host) for the general nonzero-bias case."""
    g = lambda a: np.asarray(a, np.float64)
    x = g(inputs["x"]); w_pool0 = g(inputs["w_pool0"]); b_pool0 = g(inputs["b_pool0"])
    adj1 = g(inputs["adj1"]); w_conv1 = g(inputs["w_conv1"]); b_conv1 = g(inputs["b_conv1"])
    w_q = g(inputs["w_q"]); b_q = g(inputs["b_q"])
    w_k = g(inputs["w_k"]); b_k = g(inputs["b_k"])
    alpha1 = float(g(inputs["alpha1"]).reshape(-1)[0])
    gamma = g(inputs["gamma"]); beta = g(inputs["beta"])
    w_pool1 = g(inputs["w_pool1"]); b_pool1 = float(g(inputs["b_pool1"]).reshape(-1)[0])
    w_cls = g(inputs["w_cls"]); b_cls = g(inputs["b_cls"])
    hs = np.einsum("bnc,jn->bcj", x, w_pool0) + b_pool0
    q1 = (np.einsum("bcj,qc->bqj", hs, w_q) + b_q[None, :, None]).mean(axis=1)
    k1 = (np.einsum("bcj,qc->bqj", hs, w_k) + b_k[None, :, None]).mean(axis=1)
    A1 = adj1 + np.tanh(q1[:, :, None] - k1[:, None, :]) * alpha1
    hs = np.einsum("bcj,oc->boj", hs, w_conv1) + b_conv1[None, :, None]
    hs = np.einsum("bcj,bjk->bck", hs, A1)
    mean = hs.mean(axis=(0, 2), keepdims=True)
    var = hs.var(axis=(0, 2), keepdims=True)
    hs = (hs - mean) / np.sqrt(var + BN_EPS)
    hs = hs * gamma[None, :, None] + beta[None, :, None]
    hs = (np.einsum("bcj,oj->bco", hs, w_pool1) + b_pool1).reshape(hs.shape[0], -1)
    return (hs @ w_cls.T + b_cls).astype(np.float32)


def kernel(**inputs) -> np.ndarray:
    global LAST_RESULTS
    x = np.ascontiguousarray(np.asarray(inputs["x"], dtype=np.float32))
    assert x.shape == (B, N, C), x.shape
    if not _biases_zero(inputs):
        return _numpy_reference(inputs)
    # n = p*16 + ch layout: x[b].reshape(128, 16, 512) is already [p, ch, c]
    x_bf = x.astype(_BF).reshape(B, 128, NCH * 512)
    params = _prep_params(inputs)

    nc = _built()
    in_maps = []
    for core in range(NCORES):
        m = {"x": x_bf[core * BL:(core + 1) * BL]}
        m.update(params)
        in_maps.append(m)

    trace = bool(int(os.environ.get("KERNEL_TRACE", "0")))
    res = run_bass_kernel_spmd(nc, in_maps, core_ids=list(range(NCORES)),
                               trace=trace)
    LAST_RESULTS = res

    p = np.zeros((B, C), np.float64)
    bn_sum = np.zeros(C, np.float64)
    bn_ssq = np.zeros(C, np.float64)
    for core in range(NCORES):
        out = res.results[core]
        p[core * BL:(core + 1) * BL] = np.asarray(
            out["p_out"], np.float64).reshape(BL, C)
        stats = np.asarray(out["stats_out"], np.float64)   # [2, C]
        bn_sum += stats[0]
        bn_ssq += stats[1]

    gamma = np.asarray(inputs["gamma"], np.float64)
    beta = np.asarray(inputs["beta"], np.float64)
    w1 = np.asarray(inputs["w_pool1"], np.float64)[0]
    b_pool1 = float(np.asarray(inputs["b_pool1"]).reshape(-1)[0])
    w_cls = np.asarray(inputs["w_cls"], np.float64)
    b_cls = np.asarray(inputs["b_cls"], np.float64)

    cnt = B * J
    mu = bn_sum / cnt
    var = bn_ssq / cnt - mu ** 2
    r = 1.0 / np.sqrt(var + BN_EPS)
    a = gamma * r
    S = w1.sum()
    d = beta * S + b_pool1 - a * mu * S
    out = (p * a[None, :]) @ w_cls.T + (w_cls @ d + b_cls)[None, :]
    return out.astype(np.float32)


# revision 34
# speedup vs baseline: 1.1121x; 1.0722x over previous
"""Trainium2 Bass kernel for the gnn_message_passing Combiner model.

Strategy (8 NeuronCores, data-parallel over batch, sync-BN on host):
  - batch 128 split 16-per-core, processed as 8 batch-PAIRS; params replicated.
  - x host-prepped to [BL, 128p, 16ch*512c] bf16 with n = p*16 + ch so each
    per-batch DMA is one contiguous 2 MiB transfer (16 KiB per partition);
    x streams on the SP HWDGE ring, params/outputs on the Activation ring.
  - per pair (b0, b1), all-bf16 PE path, f32 PSUM, 3-stage pipeline
    [p1 | tr+conv+qk+a1-chain | bmm]:
      p1   : hsT[128(j b0|j b1), 512c] via col-tiled matmuls — b0 at array
             cols 0-63, b1 at cols 64-127, interleaved accumulation groups
             (per-partition has_written), so the two streams run concurrently
             at full 128x128 array utilization (~216 ns per chunk pair).
      tr   : 4 PE transposes [128,128] -> hs[c, (j b0|j b1)]; evac on DVE.
      conv : 4 full-M matmuls, out [128, 512] = (hs2T b0 | hs2T b1) rows.
      qk   : 4 tiny matmuls (N=2) on the hs chunks -> q1/k1 per partition.
      krep : k1 row replicated on DVE + two concurrent quadrant transposes;
             ONE pair-wide tanh on ACT; a1 = [adj + a*T | v1 | s1] on DVE.
             (tensor_tensor_reduce hangs on HW - bisected; avoided.)
      bmm  : per batch, M=64 matmul (array cols 0-63) + M=2 extra rows
             (pooled p, channel sum) at cols 64-95; the previous pair's
             sum-of-squares ones-matmul rides at cols 96-127 concurrently.
  - outputs per core: pooled pre-BN p [8, 1024], bn stats [2, 512].
  - host: combine BN stats across cores, fold BN affine into classifier,
    tiny [128,512]@[512,200] matmul in f64.
"""

import functools
import os
from contextlib import ExitStack

import numpy as np
import ml_dtypes
_BF = ml_dtypes.bfloat16

import concourse.bass as bass
from concourse import bacc
import concourse.mybir as mybir
import concourse.tile as tile
from concourse.bass_utils import run_bass_kernel_spmd

F32 = mybir.dt.float32
BF16 = mybir.dt.bfloat16

B, N, C, J, K = 128, 2048, 512, 64, 200
NCORES = 8
BL = B // NCORES          # 16 local batches
NP = BL // 2              # 8 pairs
NCH = N // 128            # 16 n-chunks
CCH = C // 128            # 4 c-chunks
BN_EPS = 1e-5
PF = 3                    # pairs of x tiles prefetched ahead of compute

LAST_RESULTS = None       # test.py reads .exec_time_ns after a traced run


def _install_ntff_hook_shim():
    """The agent image's ``antenv`` lacks ``axon_hooks``; provide it so
    run_bass_kernel_spmd(trace=True) can capture NTFF profiles via the
    libaxon_pjrt.so C ABI (same mechanism as trn_boot's installer)."""
    import contextlib
    import ctypes
    import sys
    import types

    try:
        import antenv.axon_hooks  # noqa: F401
        return
    except ImportError:
        pass

    mod = types.ModuleType("antenv.axon_hooks")
    holder = {"hook": None}
    mod.set_axon_ntff_profile_hook = lambda h: holder.__setitem__("hook", h)
    mod.get_axon_ntff_profile_hook = lambda: holder["hook"]
    sys.modules["antenv.axon_hooks"] = mod
    try:
        import antenv
        antenv.axon_hooks = mod
    except ImportError:
        pass

    so_path = "/opt/axon/libaxon_pjrt.so"
    if not os.path.exists(so_path):
        return
    try:
        lib = ctypes.CDLL(so_path)
    except OSError:
        return
    if not hasattr(lib, "axon_start_nrt_profile"):
        return
    lib.axon_start_nrt_profile.argtypes = [
        ctypes.POINTER(ctypes.c_int64), ctypes.c_size_t]
    lib.axon_start_nrt_profile.restype = ctypes.c_int64
    lib.axon_stop_nrt_profile.argtypes = [ctypes.c_char_p]
    lib.axon_stop_nrt_profile.restype = ctypes.c_int64

    @contextlib.contextmanager
    def _hook(output_dir, device_ids):
        import jax
        jax.devices()
        if device_ids:
            ids = (ctypes.c_int64 * len(device_ids))(*device_ids)
            rc = lib.axon_start_nrt_profile(ids, len(device_ids))
        else:
            rc = lib.axon_start_nrt_profile(None, 0)
        if rc != 0:
            raise RuntimeError(f"axon_start_nrt_profile rc={rc}")
        try:
            yield
        finally:
            n = lib.axon_stop_nrt_profile(str(output_dir).encode())
            if n < 0:
                raise RuntimeError(f"axon_stop_nrt_profile rc={n}")
            print(f"profile: {n} file(s) written to {output_dir}")

    mod.set_axon_ntff_profile_hook(_hook)


_install_ntff_hook_shim()

_F = lambda k: bool(int(os.environ.get(k, "0")))
SAFE_P1 = _F("SAFE_P1")        # no b0/b1 interleave in p1
SAFE_SSQ = _F("SAFE_SSQ")      # ssq at default tile position (serialized)
SAFE_BMM = _F("SAFE_BMM")      # single M=66 bmm matmul
# tensor_tensor_reduce hangs on HW (verified by bisection) — the ttr-free
# paths are permanent defaults; env can re-enable for experiments.
SAFE_CHAIN = bool(int(os.environ.get("SAFE_CHAIN", "1")))
SAFE_DMA = _F("SAFE_DMA")      # all DMAs on the SP (sync) ring
SAFE_QK = bool(int(os.environ.get("SAFE_QK", "1")))

ADD = mybir.AluOpType.add
MULT = mybir.AluOpType.mult
TANH = mybir.ActivationFunctionType.Tanh
COPY = mybir.ActivationFunctionType.Copy
SQUARE = mybir.ActivationFunctionType.Square


class _Stage:
    """Per-pair emission state shared between pipeline stages."""
    def __init__(self):
        self.hsT_bf = None
        self.hs = None
        self.hs2T = None
        self.qk = None
        self.tr_ps = None   # [128, 640] bf16 psum: 0:512 transposes, 512:640 krepT
        self.a1 = None
        self.sq = None


def _emit_p1(nc, pools, sb, st, xt0, xt1, first):
    """hsT pair: rows 0:64 = b0's [j, c], 64:128 = b1's.  Col-tiled: b0 on
    array cols 0-63, b1 on cols 64-127, interleaved so they run concurrently.
    For the first pair, emit b0's chunks before b1's (b1's x lands later)."""
    work, psum = pools
    ps = psum.tile([128, C], F32, tag="p1", bufs=2, name="ps_p1")
    if first or SAFE_P1:
        for ch in range(NCH):
            nc.tensor.matmul(ps[0:64, :], lhsT=sb["w0"][:, ch, :],
                             rhs=xt0[:, ch * 512:(ch + 1) * 512],
                             start=(ch == 0), stop=(ch == NCH - 1))
        for ch in range(NCH):
            nc.tensor.matmul(ps[64:128, :], lhsT=sb["w0"][:, ch, :],
                             rhs=xt1[:, ch * 512:(ch + 1) * 512],
                             start=(ch == 0), stop=(ch == NCH - 1))
    else:
        # Two interleaved accumulation groups, one per partition half of the
        # same bank: has_written clears are per written partition, so the
        # groups are independent; the sim's coarse region check is skipped.
        for ch in range(NCH):
            nc.tensor.matmul(ps[0:64, :], lhsT=sb["w0"][:, ch, :],
                             rhs=xt0[:, ch * 512:(ch + 1) * 512],
                             start=(ch == 0), stop=(ch == NCH - 1),
                             skip_group_check=True)
            nc.tensor.matmul(ps[64:128, :], lhsT=sb["w0"][:, ch, :],
                             rhs=xt1[:, ch * 512:(ch + 1) * 512],
                             start=(ch == 0), stop=(ch == NCH - 1),
                             skip_group_check=True)
    hsT_bf = work.tile([128, C], BF16, tag="hsT_bf", bufs=3, name="hsT_bf")
    nc.vector.tensor_copy(hsT_bf, ps)
    st.hsT_bf = hsT_bf


def _emit_tr(nc, pools, sb, st):
    """transpose -> hs[c, (j b0 | j b1)] in 4 chunks of [128, 128] (PE)."""
    work, psum = pools
    tr_ps = psum.tile([128, 576], BF16, tag="tr", bufs=1, name="ps_tr")
    for cc in range(CCH):
        nc.tensor.transpose(tr_ps[:, cc * 128:(cc + 1) * 128],
                            in_=st.hsT_bf[:, cc * 128:(cc + 1) * 128],
                            identity=sb["ident"])
    hs = work.tile([128, C], BF16, tag="hs", bufs=3, name="hs")
    nc.vector.tensor_copy(hs, tr_ps[:, 0:512])
    st.hs = hs
    st.tr_ps = tr_ps


def _emit_convqk(nc, pools, sb, st):
    """conv1 (full-M matmuls) + qk (tiny matmuls) + tanh/a1 chain."""
    work, psum = pools
    # qk first: its DVE evac + krep2 build run while the PE does conv,
    # so the krepT LDWEIGHTS (stationary = krep2) doesn't stall the PE.
    ps_qk = psum.tile([128, 2], F32, tag="qk", bufs=1, name="ps_qk")
    for cc in range(CCH):
        nc.tensor.matmul(ps_qk, lhsT=st.hs[:, cc * 128:(cc + 1) * 128],
                         rhs=sb["wqk"][:, cc, :],
                         start=(cc == 0), stop=(cc == CCH - 1))
    qk = work.tile([128, 2], F32, tag="qk_sb", bufs=2, name="qk_sb")
    nc.vector.tensor_copy(qk, ps_qk)
    st.qk = qk
    krep2 = work.tile([128, J], BF16, tag="krep2", bufs=2, name="krep2")
    nc.vector.tensor_scalar(krep2, sb["ones128"][:, 0:J], qk[:, 1:2],
                            None, MULT)

    ps = psum.tile([128, C], F32, tag="conv", bufs=1, name="ps_conv")
    for cc in range(CCH):
        nc.tensor.matmul(ps, lhsT=st.hs[:, cc * 128:(cc + 1) * 128],
                         rhs=sb["wc"][:, cc, :],
                         start=(cc == 0), stop=(cc == CCH - 1))
    hs2T = work.tile([128, C], BF16, tag="hs2T", bufs=3, name="hs2T")
    nc.vector.tensor_copy(hs2T, ps)
    st.hs2T = hs2T

    # k1 row transposes: disjoint array quadrants, run concurrently.
    nc.tensor.matmul(st.tr_ps[0:64, 512:576], lhsT=krep2[0:64, :],
                     rhs=sb["ident2"][0:64, :], is_transpose=True,
                     skip_group_check=True)
    nc.tensor.matmul(st.tr_ps[64:128, 512:576], lhsT=krep2[64:128, :],
                     rhs=sb["ident2"][64:128, :], is_transpose=True,
                     skip_group_check=True)
    t2 = work.tile([128, J], BF16, tag="t2", bufs=2, name="t2")
    nc.scalar.activation(t2, st.tr_ps[:, 512:576], TANH,
                         bias=qk[:, 0:1], scale=-1.0)
    t2a = work.tile([128, J], BF16, tag="t2a", bufs=2, name="t2a")
    nc.vector.tensor_scalar(t2a, t2, sb["alpha"], None, MULT)
    s1t = work.tile([128, 1], F32, tag="s1t", bufs=2, name="s1t")
    nc.vector.tensor_reduce(s1t, t2a, axis=mybir.AxisListType.X, op=ADD)
    a1 = work.tile([128, J + 2], BF16, tag="a1", bufs=2, name="a1")
    nc.vector.tensor_tensor(a1[:, 0:J], t2a, sb["adj"], op=ADD)
    tw = work.tile([128, J], BF16, tag="tw", bufs=1, name="tw")
    nc.vector.tensor_tensor(tw, t2a, sb["w1rep"], op=MULT)
    v1t = work.tile([128, 1], F32, tag="v1t", bufs=2, name="v1t")
    nc.vector.tensor_reduce(v1t, tw, axis=mybir.AxisListType.X, op=ADD)
    nc.vector.tensor_scalar(a1[:, J:J + 1], v1t, sb["adjv"], None, ADD)
    nc.vector.tensor_scalar(a1[:, J + 1:J + 2], s1t, sb["adjs"], None, ADD)
    st.a1 = a1


def _emit_bmm(nc, pools, sb, st, pp, st_prev, psum_ssq, p_pair):
    """Per batch of the pair: M=64 bmm (array cols 0-63) + M=2 extra rows
    (cols 64-95); the previous pair's ssq ones-matmul rides at cols 96-127.
    sq/p4/bn2 evacuations all on DVE (ACT does only tanh)."""
    work, psum = pools
    sq = work.tile([128, C], BF16, tag="sq", bufs=3, name="sq")
    for b in range(2):
        base = 64 * b
        pb = psum.tile([66, C], F32, tag="bmm", bufs=2, name="ps_bmm")
        if SAFE_BMM:
            nc.tensor.matmul(pb, lhsT=st.a1[base:base + 64, :],
                             rhs=st.hs2T[base:base + 64, :],
                             start=True, stop=True)
        else:
            nc.tensor.matmul(pb[0:64, :], lhsT=st.a1[base:base + 64, 0:64],
                             rhs=st.hs2T[base:base + 64, :],
                             start=True, stop=True)
            nc.tensor.matmul(pb[64:66, :], lhsT=st.a1[base:base + 64, 64:66],
                             rhs=st.hs2T[base:base + 64, :],
                             start=True, stop=True, skip_group_check=True)
        if b == 0 and st_prev is not None:
            _emit_ssq(nc, sb, st_prev, pp - 1, psum_ssq)
        nc.scalar.activation(sq[base:base + 64, :], pb[0:64, :], SQUARE)
        nc.scalar.activation(p_pair[:, b * C:(b + 1) * C], pb[64:65, :], COPY)
        nc.vector.tensor_tensor(sb["bn2"], sb["bn2"], pb[64:66, :], op=ADD)
    st.sq = sq


def _emit_ssq(nc, sb, st, pp, psum_ssq):
    """Accumulate sum-of-squares over (pair, j) into one PSUM row via PE,
    col-tiled at array cols 96-127 so it overlaps the adjacent bmm."""
    if SAFE_SSQ:
        nc.tensor.matmul(psum_ssq[0:1, :], lhsT=sb["onescol"], rhs=st.sq,
                         start=(pp == 0), stop=(pp == NP - 1))
    else:
        nc.tensor.matmul(psum_ssq[96:97, :], lhsT=sb["onescol"], rhs=st.sq,
                         start=(pp == 0), stop=(pp == NP - 1),
                         tile_position=(0, 96))


def _build():
    nc = bacc.Bacc("TRN2", target_bir_lowering=False)

    x = nc.dram_tensor("x", [BL, 128, NCH * 512], BF16, kind="ExternalInput")
    w0p = nc.dram_tensor("w0p", [128, NCH * J], BF16, kind="ExternalInput")
    wcp = nc.dram_tensor("wcp", [128, CCH * C], BF16, kind="ExternalInput")
    wqkp = nc.dram_tensor("wqkp", [128, CCH * 2], BF16, kind="ExternalInput")
    adj2 = nc.dram_tensor("adj2", [128, J], F32, kind="ExternalInput")
    alpha2 = nc.dram_tensor("alpha2", [128, 1], F32, kind="ExternalInput")
    adjv2 = nc.dram_tensor("adjv2", [128, 1], F32, kind="ExternalInput")
    adjs2 = nc.dram_tensor("adjs2", [128, 1], F32, kind="ExternalInput")
    w1rep2 = nc.dram_tensor("w1rep2", [128, J], F32, kind="ExternalInput")

    p_out = nc.dram_tensor("p_out", [NP, 2 * C], F32, kind="ExternalOutput")
    stats_out = nc.dram_tensor("stats_out", [2, C], F32, kind="ExternalOutput")

    pdma = nc.sync if SAFE_DMA else nc.scalar

    with ExitStack() as ctx:
        tc = ctx.enter_context(tile.TileContext(nc))
        consts = ctx.enter_context(tc.tile_pool(name="consts", bufs=1))
        xpool = ctx.enter_context(tc.tile_pool(name="xpool", bufs=2 * (PF + 1)))
        work = ctx.enter_context(tc.tile_pool(name="work", bufs=2))
        psum = ctx.enter_context(tc.tile_pool(name="psum", bufs=1, space="PSUM"))

        # ---- constants on the Activation HWDGE ring (parallel to x) ----
        w0_sb = consts.tile([128, NCH, J], BF16, name="w0_sb")
        pdma.dma_start(out=w0_sb, in_=w0p.rearrange("p (t j) -> p t j", j=J))
        ident_dram = nc.inline_tensor(
            np.eye(128, dtype=np.float32).astype(_BF), name="ident128")
        ident = consts.tile([128, 128], BF16, name="ident")
        pdma.dma_start(out=ident, in_=ident_dram[:, :])
        wqk_sb = consts.tile([128, CCH, 2], BF16, name="wqk_sb")
        pdma.dma_start(out=wqk_sb, in_=wqkp.rearrange("p (q s) -> p q s", s=2))
        ident2_dram = nc.inline_tensor(
            np.tile(np.eye(J, dtype=np.float32), (2, 1)).astype(_BF),
            name="identj2")
        ident2 = consts.tile([128, J], BF16, name="ident2")
        pdma.dma_start(out=ident2, in_=ident2_dram[:, :])
        adj_sb = consts.tile([128, J], F32, name="adj_sb")
        pdma.dma_start(out=adj_sb, in_=adj2[:, :])
        alpha_sb = consts.tile([128, 1], F32, name="alpha_sb")
        pdma.dma_start(out=alpha_sb, in_=alpha2[:, :])
        adjv_sb = consts.tile([128, 1], F32, name="adjv_sb")
        pdma.dma_start(out=adjv_sb, in_=adjv2[:, :])
        adjs_sb = consts.tile([128, 1], F32, name="adjs_sb")
        pdma.dma_start(out=adjs_sb, in_=adjs2[:, :])
        w1rep_sb = consts.tile([128, J], F32, name="w1rep_sb")
        pdma.dma_start(out=w1rep_sb, in_=w1rep2[:, :])
        wc_sb = consts.tile([128, CCH, C], BF16, name="wc_sb")
        pdma.dma_start(out=wc_sb, in_=wcp.rearrange("p (q o) -> p q o", o=C))

        ones128 = consts.tile([128, 128], BF16, name="ones128")
        nc.vector.memset(ones128, 1.0)
        onescol = consts.tile([128, 1], BF16, name="onescol")
        nc.vector.memset(onescol, 1.0)
        bn2 = consts.tile([2, C], F32, name="bn2")
        nc.vector.memset(bn2, 0.0)

        # ---- x prefetch ring on the SP ring (one 2 MiB DMA per batch;
        # the first two batches split finer to shorten the ramp) ----
        xts = {}

        def load_x(b):
            if b < BL:
                xt = xpool.tile([128, NCH * 512], BF16, tag="xt", name="xt")
                nsplit = 8 if b == 0 else (4 if b == 1 else 1)
                q = NCH * 512 // nsplit
                for i in range(nsplit):
                    nc.sync.dma_start(out=xt[:, i * q:(i + 1) * q],
                                      in_=x[b, :, i * q:(i + 1) * q])
                xts[b] = xt

        for b in range(2 * PF):
            load_x(b)

        sb = dict(w0=w0_sb, wc=wc_sb, wqk=wqk_sb, ident2=ident2,
                  adj=adj_sb, alpha=alpha_sb, adjv=adjv_sb, adjs=adjs_sb,
                  w1rep=w1rep_sb, ident=ident, ones128=ones128,
                  onescol=onescol, bn2=bn2)
        pools = (work, psum)

        psum_ssq = psum.tile([128, C], F32, tag="ssq", bufs=1, name="ps_ssq")
        stages = [_Stage() for _ in range(NP)]
        p_pairs = [None] * NP

        def emit_bmm_stage(pp):
            p_pairs[pp] = pp_t = work.tile([1, 2 * C], F32, tag="p_pair",
                                           bufs=2, name="p_pair")
            _emit_bmm(nc, pools, sb, stages[pp], pp,
                      stages[pp - 1] if pp >= 1 else None, psum_ssq, pp_t)
            pdma.dma_start(out=p_out[pp:pp + 1, :], in_=pp_t)

        for i in range(NP):
            load_x(2 * (i + PF))
            load_x(2 * (i + PF) + 1)
            _emit_p1(nc, pools, sb, stages[i], xts.pop(2 * i),
                     xts.pop(2 * i + 1), first=(i == 0))
            if i >= 2:
                emit_bmm_stage(i - 2)
            if i >= 1:
                _emit_tr(nc, pools, sb, stages[i - 1])
                _emit_convqk(nc, pools, sb, stages[i - 1])

        # drain
        _emit_tr(nc, pools, sb, stages[NP - 1])
        _emit_convqk(nc, pools, sb, stages[NP - 1])
        emit_bmm_stage(NP - 2)
        emit_bmm_stage(NP - 1)
        _emit_ssq(nc, sb, stages[NP - 1], NP - 1, psum_ssq)

        # final stats on both HWDGE rings so the two DMAs run in parallel
        # (the SP ring is idle once the x stream has finished).
        ssq_row = 0 if SAFE_SSQ else 96
        ssq_sb = consts.tile([97, C], F32, name="ssq_sb")
        nc.vector.tensor_copy(ssq_sb[ssq_row:ssq_row + 1, :],
                              psum_ssq[ssq_row:ssq_row + 1, :])
        nc.sync.dma_start(out=stats_out[0:1, :], in_=bn2[1:2, :])
        pdma.dma_start(out=stats_out[1:2, :],
                       in_=ssq_sb[ssq_row:ssq_row + 1, :])

    nc.compile()
    return nc


@functools.lru_cache(maxsize=1)
def _built():
    return _build()


def _prep_params(inputs):
    f = lambda a: np.ascontiguousarray(np.asarray(a, dtype=np.float32))
    w_pool0 = f(inputs["w_pool0"])                       # [J, N]
    w0p = np.ascontiguousarray(
        w_pool0.reshape(J, 128, NCH).transpose(1, 2, 0)  # [p, ch, j]
    ).reshape(128, NCH * J).astype(_BF)
    w_conv1 = f(inputs["w_conv1"])                       # [O, C]
    wcp = np.ascontiguousarray(
        w_conv1.T.reshape(CCH, 128, C).transpose(1, 0, 2)  # [p, cc, o]
    ).reshape(128, CCH * C).astype(_BF)
    w_q, w_k = f(inputs["w_q"]), f(inputs["w_k"])
    wqk = np.stack([w_q.mean(axis=0), w_k.mean(axis=0)], axis=1)  # [C, 2]
    wqkp = np.ascontiguousarray(
        wqk.reshape(CCH, 128, 2).transpose(1, 0, 2)
    ).reshape(128, CCH * 2).astype(_BF)
    adj1 = np.asarray(inputs["adj1"], np.float64)
    w1 = np.asarray(inputs["w_pool1"], np.float64).reshape(J)
    t2 = lambda a: np.ascontiguousarray(np.tile(a, (2, 1))).astype(np.float32)
    params = {
        "w0p": w0p, "wcp": wcp, "wqkp": wqkp,
        "adj2": t2(np.asarray(inputs["adj1"], np.float32)),
        "alpha2": np.full((128, 1),
                          np.asarray(inputs["alpha1"]).reshape(-1)[0],
                          np.float32),
        "adjv2": t2((adj1 @ w1).astype(np.float32).reshape(J, 1)),
        "adjs2": t2(adj1.sum(axis=1).astype(np.float32).reshape(J, 1)),
        "w1rep2": np.tile(w1.astype(np.float32)[None, :], (128, 1)),
    }
    return params


def _biases_zero(inputs):
    return all(np.abs(np.asarray(inputs[k])).max() < 1e-30
               for k in ("b_pool0", "b_conv1", "b_q", "b_k"))


def _numpy_reference(inputs):
    """Exact fallback (host) for the general nonzero-bias case."""
    g = lambda a: np.asarray(a, np.float64)
    x = g(inputs["x"]); w_pool0 = g(inputs["w_pool0"]); b_pool0 = g(inputs["b_pool0"])
    adj1 = g(inputs["adj1"]); w_conv1 = g(inputs["w_conv1"]); b_conv1 = g(inputs["b_conv1"])
    w_q = g(inputs["w_q"]); b_q = g(inputs["b_q"])
    w_k = g(inputs["w_k"]); b_k = g(inputs["b_k"])
    alpha1 = float(g(inputs["alpha1"]).reshape(-1)[0])
    gamma = g(inputs["gamma"]); beta = g(inputs["beta"])
    w_pool1 = g(inputs["w_pool1"]); b_pool1 = float(g(inputs["b_pool1"]).reshape(-1)[0])
    w_cls = g(inputs["w_cls"]); b_cls = g(inputs["b_cls"])
    hs = np.einsum("bnc,jn->bcj", x, w_pool0) + b_pool0
    q1 = (np.einsum("bcj,qc->bqj", hs, w_q) + b_q[None, :, None]).mean(axis=1)
    k1 = (np.einsum("bcj,qc->bqj", hs, w_k) + b_k[None, :, None]).mean(axis=1)
    A1 = adj1 + np.tanh(q1[:, :, None] - k1[:, None, :]) * alpha1
    hs = np.einsum("bcj,oc->boj", hs, w_conv1) + b_conv1[None, :, None]
    hs = np.einsum("bcj,bjk->bck", hs, A1)
    mean = hs.mean(axis=(0, 2), keepdims=True)
    var = hs.var(axis=(0, 2), keepdims=True)
    hs = (hs - mean) / np.sqrt(var + BN_EPS)
    hs = hs * gamma[None, :, None] + beta[None, :, None]
    hs = (np.einsum("bcj,oj->bco", hs, w_pool1) + b_pool1).reshape(hs.shape[0], -1)
    return (hs @ w_cls.T + b_cls).astype(np.float32)


def kernel(**inputs) -> np.ndarray:
    global LAST_RESULTS
    x = np.ascontiguousarray(np.asarray(inputs["x"], dtype=np.float32))
    assert x.shape == (B, N, C), x.shape
    if not _biases_zero(inputs):
        return _numpy_reference(inputs)
    # n = p*16 + ch layout: x[b].reshape(128, 16, 512) is already [p, ch, c]
    x_bf = x.astype(_BF).reshape(B, 128, NCH * 512)
    params = _prep_params(inputs)

    nc = _built()
    in_maps = []
    for core in range(NCORES):
        m = {"x": x_bf[core * BL:(core + 1) * BL]}
        m.update(params)
        in_maps.append(m)

    trace = bool(int(os.environ.get("KERNEL_TRACE", "0")))
    res = run_bass_kernel_spmd(nc, in_maps, core_ids=list(range(NCORES)),
                               trace=trace)
    LAST_RESULTS = res

    p = np.zeros((B, C), np.float64)
    bn_sum = np.zeros(C, np.float64)
    bn_ssq = np.zeros(C, np.float64)
    for core in range(NCORES):
        out = res.results[core]
        p[core * BL:(core + 1) * BL] = np.asarray(
            out["p_out"], np.float64).reshape(BL, C)
        stats = np.asarray(out["stats_out"], np.float64)   # [2, C]
        bn_sum += stats[0]
        bn_ssq += stats[1]

    gamma = np.asarray(inputs["gamma"], np.float64)
    beta = np.asarray(inputs["beta"], np.float64)
    w1 = np.asarray(inputs["w_pool1"], np.float64)[0]
    b_pool1 = float(np.asarray(inputs["b_pool1"]).reshape(-1)[0])
    w_cls = np.asarray(inputs["w_cls"], np.float64)
    b_cls = np.asarray(inputs["b_cls"], np.float64)

    cnt = B * J
    mu = bn_sum / cnt
    var = bn_ssq / cnt - mu ** 2
    r = 1.0 / np.sqrt(var + BN_EPS)
    a = gamma * r
    S = w1.sum()
    d = beta * S + b_pool1 - a * mu * S
    out = (p * a[None, :]) @ w_cls.T + (w_cls @ d + b_cls)[None, :]
    return out.astype(np.float32)
